# revision 6
# baseline (speedup 1.0000x reference)
"""Trainium2 Bass kernel for:
    y = gelu_logistic(gelu_logistic(leaky(leaky(logsumexp(x @ W^T + b, axis=1)))))

Strategy: data-parallel over rows of x across 8 NeuronCores (2048 rows/core),
weight + bias replicated.  Per core, a bf16 matmul (PE) computes logits in
PSUM 512 columns at a time; the Scalar engine applies exp in one pass with a
fused free-dim sum (accum_out); the tiny [rows, 1] epilogue (log, leaky^2,
gelu^2) runs on Scalar/Vector engines; no collectives needed.

The bias is folded into the matmul as an extra contraction tile: x gets an
appended row of ones, W^T gets an appended row holding b.

Host-side prep (outside the timed device kernel): shard + downcast to bf16 +
retile so every DMA is a contiguous per-partition stream.
"""

import numpy as np
import ml_dtypes

import concourse.bass as bass
import concourse.tile as tile
from concourse import bacc, mybir
from concourse.bass_utils import run_bass_kernel_spmd

BF16 = ml_dtypes.bfloat16
P = 128  # partitions / contraction tile
FREE = 512  # matmul moving free dim = one PSUM bank of fp32

GELU_K = 1.702
LEAKY2 = 0.01 * 0.01  # leaky(leaky(x)) == max(x, (0.01^2) * x)


class Cfg:
    def __init__(self, M=16384, K=4096, N=4096, n_cores=8, xw=256, nqw=1024):
        self.M, self.K, self.N, self.n_cores = M, K, N, n_cores
        self.MS = M // n_cores  # rows per core
        self.MT = self.MS // P  # m-tiles per core
        self.XW = xw  # m-columns per x DMA tile
        self.MP = self.MS // xw  # x DMA tiles per core
        self.MI = xw // P  # m-tiles per x DMA tile
        self.KT = K // P + 1  # contraction tiles (+1 bias tile)
        self.NQW = nqw  # n-columns per resident W tile
        self.NQ = N // nqw  # W tiles streamed per core
        self.NB = nqw // FREE  # matmul free blocks per W tile
        assert M % n_cores == 0 and self.MS % xw == 0 and xw % P == 0
        assert K % P == 0 and N % nqw == 0 and nqw % FREE == 0


def build(nc: bass.Bass, cfg: Cfg):
    """Emit the per-core kernel. SPMD: every core runs this same program."""
    c = cfg
    xt_d = nc.dram_tensor("xt", [c.MP, P, c.KT, c.XW], mybir.dt.bfloat16,
                          kind="ExternalInput")
    wq_d = nc.dram_tensor("wq", [c.NQ, P, c.KT, c.NQW], mybir.dt.bfloat16,
                          kind="ExternalInput")
    out_d = nc.dram_tensor("out", [c.MS, 1], mybir.dt.float32,
                           kind="ExternalOutput")

    fp32 = mybir.dt.float32
    AF = mybir.ActivationFunctionType

    with tile.TileContext(nc) as tc:
        with (
            tc.tile_pool(name="wpool", bufs=2) as wpool,
            tc.tile_pool(name="xpool", bufs=3) as xpool,
            tc.tile_pool(name="epool", bufs=3) as epool,
            tc.tile_pool(name="psum", bufs=8, space="PSUM") as psum,
            tc.tile_pool(name="accp", bufs=1) as accp,
        ):
            # per-(m-tile, n-chunk) partial sums of exp(logits)
            acc = accp.tile([P, c.MT, c.NQ * c.NB], fp32)

            for nq in range(c.NQ):
                wq = wpool.tile([P, c.KT, c.NQW], mybir.dt.bfloat16)
                nc.sync.dma_start(wq[:], wq_d[nq])
                for mp in range(c.MP):
                    xp = xpool.tile([P, c.KT, c.XW], mybir.dt.bfloat16)
                    nc.sync.dma_start(xp[:], xt_d[mp])
                    for mi in range(c.MI):
                        mt = mp * c.MI + mi
                        pts = [psum.tile([P, FREE], fp32, name=f"ps{nb}",
                                         tag="ps")
                               for nb in range(c.NB)]
                        for k in range(c.KT):
                            lhsT = xp[:, k, mi * P:(mi + 1) * P]
                            for nb in range(c.NB):
                                nc.tensor.matmul(
                                    pts[nb][:],
                                    lhsT,
                                    wq[:, k, nb * FREE:(nb + 1) * FREE],
                                    start=(k == 0),
                                    stop=(k == c.KT - 1),
                                )
                        # exp + row-sum in one Scalar-engine pass per block
                        for nb in range(c.NB):
                            scratch = epool.tile([P, FREE], fp32, tag="exps")
                            col = nq * c.NB + nb
                            nc.scalar.activation(
                                scratch[:], pts[nb][:], AF.Exp,
                                accum_out=acc[:, mt, col:col + 1],
                            )

            # ---- epilogue on [P, MT] (tiny) ----
            S = accp.tile([P, c.MT], fp32)
            t = accp.tile([P, c.MT], fp32)
            for mt in range(c.MT):
                nc.vector.tensor_reduce(
                    S[:, mt:mt + 1], acc[:, mt, :],
                    axis=mybir.AxisListType.X, op=mybir.AluOpType.add,
                )
            # lse = ln(sum exp)  (logits are O(5); no max-shift needed)
            nc.scalar.activation(S[:], S[:], AF.Ln)
            # leaky(leaky(x)) = max(x, 1e-4 x)
            nc.vector.tensor_scalar_mul(t[:], S[:], LEAKY2)
            nc.vector.tensor_max(S[:], S[:], t[:])
            # gelu_logistic(x) = x * sigmoid(1.702 x), applied twice
            nc.scalar.activation(t[:], S[:], AF.Sigmoid, scale=GELU_K)
            nc.vector.tensor_mul(S[:], S[:], t[:])
            nc.scalar.activation(t[:], S[:], AF.Sigmoid, scale=GELU_K)
            nc.vector.tensor_mul(S[:], S[:], t[:])

            out_v = out_d[:].rearrange("(t p) o -> t p o", p=P)
            for mt in range(c.MT):
                nc.sync.dma_start(out_v[mt], S[:, mt:mt + 1])
    return nc


def prep_w(weight: np.ndarray, bias: np.ndarray, cfg: Cfg) -> np.ndarray:
    """[N, K] weight + [N] bias -> [NQ, P, KT, NQW] bf16 with bias row."""
    c = cfg
    kt0 = c.K // P
    wb = np.ascontiguousarray(weight.astype(BF16))  # [N, K]
    wq = np.zeros((c.NQ, P, c.KT, c.NQW), dtype=BF16)
    # wq[nq, p, k, n] = W[nq*NQW + n, k*P + p]
    wq[:, :, :kt0, :] = (
        wb.reshape(c.NQ, c.NQW, kt0, P).transpose(0, 3, 2, 1)
    )
    wq[:, 0, kt0, :] = bias.astype(BF16).reshape(c.NQ, c.NQW)
    return wq


def prep_x_shard(xs: np.ndarray, cfg: Cfg) -> np.ndarray:
    """[MS, K] fp32 shard -> [MP, P, KT, XW] bf16 with ones row."""
    c = cfg
    kt0 = c.K // P
    xb = xs.astype(BF16)  # [MS, K]
    xt = np.zeros((c.MP, P, c.KT, c.XW), dtype=BF16)
    # xt[mp, p, k, m] = x[mp*XW + m, k*P + p]
    xt[:, :, :kt0, :] = (
        xb.reshape(c.MP, c.XW, kt0, P).transpose(0, 3, 2, 1)
    )
    xt[:, 0, kt0, :] = 1.0
    return xt


_BUILT = {}


def _get_built(cfg_key=None):
    if cfg_key is None:
        cfg = Cfg()
    else:
        cfg = cfg_key
    key = (cfg.M, cfg.K, cfg.N, cfg.n_cores)
    if key not in _BUILT:
        nc = bacc.Bacc("TRN2")
        build(nc, cfg)
        nc.compile()
        _BUILT[key] = (nc, cfg)
    return _BUILT[key]


def _install_ntff_hook():
    """Dev-only: register the axon NTFF profile hook that the container's
    antenv stub lacks, so trace=True works. No-op if unavailable."""
    import sys
    import types
    try:
        from antenv.axon_hooks import get_axon_ntff_profile_hook  # noqa: F401
        return
    except ImportError:
        pass
    try:
        import antenv
        from trn_agent_boot.trn_boot import _ntff_profile_via_ctypes
        mod = types.ModuleType("antenv.axon_hooks")
        holder = {}
        mod.set_axon_ntff_profile_hook = lambda h: holder.__setitem__("h", h)
        mod.get_axon_ntff_profile_hook = lambda: holder.get("h")
        sys.modules["antenv.axon_hooks"] = mod
        antenv.axon_hooks = mod
        hook = _ntff_profile_via_ctypes("/opt/axon/libaxon_pjrt.so")
        if hook is not None:
            mod.set_axon_ntff_profile_hook(hook)
    except Exception as e:  # pragma: no cover - best effort
        print(f"ntff hook install failed: {e}", file=sys.stderr)


def run(x, weight, bias, trace=False):
    """Full-input entry: shard, run on 8 cores, gather. Returns
    (out [M,1] fp32, exec_time_ns or None, trace_path or None)."""
    if trace:
        _install_ntff_hook()
    nc, cfg = _get_built()
    x = np.asarray(x, dtype=np.float32)
    weight = np.asarray(weight, dtype=np.float32)
    bias = np.asarray(bias, dtype=np.float32)

    wq = prep_w(weight, bias, cfg)
    in_maps = []
    for core in range(cfg.n_cores):
        xs = x[core * cfg.MS:(core + 1) * cfg.MS]
        in_maps.append({"xt": prep_x_shard(xs, cfg), "wq": wq})

    res = run_bass_kernel_spmd(
        nc, in_maps, core_ids=list(range(cfg.n_cores)), trace=trace,
    )
    out = np.concatenate([r["out"] for r in res.results], axis=0)
    trace_path = None
    if res.instructions_and_trace is not None:
        trace_path = res.instructions_and_trace[1]
    return out, res.exec_time_ns, trace_path


def kernel(x, weight, bias):
    out, _, _ = run(x, weight, bias, trace=False)
    return out


# revision 10
# speedup vs baseline: 1.9008x; 1.9008x over previous
"""Trainium2 Bass kernel for:
    y = gelu_logistic(gelu_logistic(leaky(leaky(logsumexp(x @ W^T + b, axis=1)))))

Strategy: data-parallel over rows of x across 8 NeuronCores (2048 rows/core),
weight + bias replicated.  Per core, a bf16 matmul (PE) computes logits in
PSUM 512 columns at a time; the Scalar engine applies exp in one pass with a
fused free-dim sum (accum_out); the tiny [rows, 1] epilogue (log, leaky^2,
gelu^2) runs on Scalar/Vector engines; no collectives needed.

The bias is folded into the matmul as an extra contraction tile: x gets an
appended row of ones, W^T gets an appended row holding b.

Host-side prep (outside the timed device kernel): shard + downcast to bf16 +
retile so every DMA is a contiguous per-partition stream.
"""

import numpy as np
import ml_dtypes

import concourse.bass as bass
import concourse.tile as tile
from concourse import bacc, mybir
from concourse.bass_utils import run_bass_kernel_spmd

BF16 = ml_dtypes.bfloat16
P = 128  # partitions / contraction tile
FREE = 512  # matmul moving free dim = one PSUM bank of fp32

GELU_K = 1.702
LEAKY2 = 0.01 * 0.01  # leaky(leaky(x)) == max(x, (0.01^2) * x)


W_SCALE = 64.0  # fp8 path: W,b scaled by 64 into e4m3 range; exp descales


class Cfg:
    def __init__(self, M=16384, K=4096, N=4096, n_cores=8, xw=256, nqw=1024,
                 dtype="bf16"):
        self.M, self.K, self.N, self.n_cores = M, K, N, n_cores
        self.dtype = dtype
        self.MS = M // n_cores  # rows per core
        self.MT = self.MS // P  # m-tiles per core
        self.XW = xw  # m-columns per x DMA tile
        self.MP = self.MS // xw  # x DMA tiles per core
        self.MI = xw // P  # m-tiles per x DMA tile
        self.KT = K // P + 1  # contraction tiles (+1 bias tile) [bf16 path]
        self.KT2 = K // (2 * P)  # DoubleRow pair tiles [fp8 path]
        self.NQW = nqw  # n-columns per resident W tile
        self.NQ = N // nqw  # W tiles streamed per core
        self.NB = nqw // FREE  # matmul free blocks per W tile
        assert M % n_cores == 0 and self.MS % xw == 0 and xw % P == 0
        assert K % P == 0 and N % nqw == 0 and nqw % FREE == 0
        if dtype == "fp8":
            assert K % (2 * P) == 0


def build(nc: bass.Bass, cfg: Cfg):
    """Emit the per-core kernel. SPMD: every core runs this same program."""
    c = cfg
    xt_d = nc.dram_tensor("xt", [c.MP, P, c.KT, c.XW], mybir.dt.bfloat16,
                          kind="ExternalInput")
    wq_d = nc.dram_tensor("wq", [c.NQ, P, c.KT, c.NQW], mybir.dt.bfloat16,
                          kind="ExternalInput")
    out_d = nc.dram_tensor("out", [c.MS, 1], mybir.dt.float32,
                           kind="ExternalOutput")

    fp32 = mybir.dt.float32
    AF = mybir.ActivationFunctionType

    with tile.TileContext(nc) as tc:
        with (
            tc.tile_pool(name="wpool", bufs=2) as wpool,
            tc.tile_pool(name="xpool", bufs=3) as xpool,
            tc.tile_pool(name="epool", bufs=3) as epool,
            tc.tile_pool(name="psum", bufs=8, space="PSUM") as psum,
            tc.tile_pool(name="accp", bufs=1) as accp,
        ):
            # per-(m-tile, n-chunk) partial sums of exp(logits)
            acc = accp.tile([P, c.MT, c.NQ * c.NB], fp32)

            for nq in range(c.NQ):
                wq = wpool.tile([P, c.KT, c.NQW], mybir.dt.bfloat16)
                nc.sync.dma_start(wq[:], wq_d[nq])
                for mp in range(c.MP):
                    xp = xpool.tile([P, c.KT, c.XW], mybir.dt.bfloat16)
                    nc.sync.dma_start(xp[:], xt_d[mp])
                    for mi in range(c.MI):
                        mt = mp * c.MI + mi
                        pts = [psum.tile([P, FREE], fp32, name=f"ps{nb}",
                                         tag="ps")
                               for nb in range(c.NB)]
                        for k in range(c.KT):
                            lhsT = xp[:, k, mi * P:(mi + 1) * P]
                            for nb in range(c.NB):
                                nc.tensor.matmul(
                                    pts[nb][:],
                                    lhsT,
                                    wq[:, k, nb * FREE:(nb + 1) * FREE],
                                    start=(k == 0),
                                    stop=(k == c.KT - 1),
                                )
                        # exp + row-sum in one Scalar-engine pass per block
                        for nb in range(c.NB):
                            scratch = epool.tile([P, FREE], fp32, tag="exps")
                            col = nq * c.NB + nb
                            nc.scalar.activation(
                                scratch[:], pts[nb][:], AF.Exp,
                                accum_out=acc[:, mt, col:col + 1],
                            )

            # ---- epilogue on [P, MT] (tiny) ----
            S = accp.tile([P, c.MT], fp32)
            t = accp.tile([P, c.MT], fp32)
            for mt in range(c.MT):
                nc.vector.tensor_reduce(
                    S[:, mt:mt + 1], acc[:, mt, :],
                    axis=mybir.AxisListType.X, op=mybir.AluOpType.add,
                )
            # lse = ln(sum exp)  (logits are O(5); no max-shift needed)
            nc.scalar.activation(S[:], S[:], AF.Ln)
            # leaky(leaky(x)) = max(x, 1e-4 x)
            nc.vector.tensor_scalar_mul(t[:], S[:], LEAKY2)
            nc.vector.tensor_max(S[:], S[:], t[:])
            # gelu_logistic(x) = x * sigmoid(1.702 x), applied twice
            nc.scalar.activation(t[:], S[:], AF.Sigmoid, scale=GELU_K)
            nc.vector.tensor_mul(S[:], S[:], t[:])
            nc.scalar.activation(t[:], S[:], AF.Sigmoid, scale=GELU_K)
            nc.vector.tensor_mul(S[:], S[:], t[:])

            out_v = out_d[:].rearrange("(t p) o -> t p o", p=P)
            for mt in range(c.MT):
                nc.sync.dma_start(out_v[mt], S[:, mt:mt + 1])
    return nc


def build_fp8(nc: bass.Bass, cfg: Cfg):
    """fp8 e4m3 DoubleRow variant: 2 contraction tiles per matmul.
    W (and bias) are pre-scaled by W_SCALE on the host; the exp's affine
    scale divides it back out. Bias is added per 512-block on VectorE."""
    c = cfg
    xt_d = nc.dram_tensor("xt", [c.MP, P, c.KT2, 2, c.XW], mybir.dt.float8e4,
                          kind="ExternalInput")
    wq_d = nc.dram_tensor("wq", [c.NQ, P, c.KT2, 2, c.NQW], mybir.dt.float8e4,
                          kind="ExternalInput")
    br_d = nc.dram_tensor("biasr", [P, c.N], mybir.dt.float32,
                          kind="ExternalInput")
    out_d = nc.dram_tensor("out", [c.MS, 1], mybir.dt.float32,
                           kind="ExternalOutput")

    fp32 = mybir.dt.float32
    AF = mybir.ActivationFunctionType
    DR = mybir.MatmulPerfMode.DoubleRow

    with tile.TileContext(nc) as tc:
        with (
            tc.tile_pool(name="wpool", bufs=2) as wpool,
            tc.tile_pool(name="xpool", bufs=3) as xpool,
            tc.tile_pool(name="epool", bufs=3) as epool,
            tc.tile_pool(name="psum", bufs=8, space="PSUM") as psum,
            tc.tile_pool(name="accp", bufs=1) as accp,
        ):
            bias_sb = accp.tile([P, c.N], fp32)
            nc.sync.dma_start(bias_sb[:], br_d[:])
            acc = accp.tile([P, c.MT, c.NQ * c.NB], fp32)

            for nq in range(c.NQ):
                wq = wpool.tile([P, c.KT2, 2, c.NQW], mybir.dt.float8e4)
                nc.sync.dma_start(wq[:], wq_d[nq])
                for mp in range(c.MP):
                    xp = xpool.tile([P, c.KT2, 2, c.XW], mybir.dt.float8e4)
                    nc.sync.dma_start(xp[:], xt_d[mp])
                    for mi in range(c.MI):
                        mt = mp * c.MI + mi
                        pts = [psum.tile([P, FREE], fp32, name=f"ps{nb}",
                                         tag="ps")
                               for nb in range(c.NB)]
                        for kk in range(c.KT2):
                            lhsT = xp[:, kk, :, mi * P:(mi + 1) * P]
                            for nb in range(c.NB):
                                nc.tensor.matmul(
                                    pts[nb][:],
                                    lhsT,
                                    wq[:, kk, :, nb * FREE:(nb + 1) * FREE],
                                    start=(kk == 0),
                                    stop=(kk == c.KT2 - 1),
                                    perf_mode=DR,
                                )
                        for nb in range(c.NB):
                            col = nq * c.NB + nb
                            n0 = col * FREE
                            # psum += W_SCALE * bias  (still in scaled units)
                            nc.vector.tensor_add(
                                pts[nb][:], pts[nb][:],
                                bias_sb[:, n0:n0 + FREE],
                            )
                            scratch = epool.tile([P, FREE], fp32, tag="exps")
                            nc.scalar.activation(
                                scratch[:], pts[nb][:], AF.Exp,
                                scale=1.0 / W_SCALE,
                                accum_out=acc[:, mt, col:col + 1],
                            )

            # ---- epilogue on [P, MT] (tiny) ----
            S = accp.tile([P, c.MT], fp32)
            t = accp.tile([P, c.MT], fp32)
            for mt in range(c.MT):
                nc.vector.tensor_reduce(
                    S[:, mt:mt + 1], acc[:, mt, :],
                    axis=mybir.AxisListType.X, op=mybir.AluOpType.add,
                )
            nc.scalar.activation(S[:], S[:], AF.Ln)
            nc.vector.tensor_scalar_mul(t[:], S[:], LEAKY2)
            nc.vector.tensor_max(S[:], S[:], t[:])
            nc.scalar.activation(t[:], S[:], AF.Sigmoid, scale=GELU_K)
            nc.vector.tensor_mul(S[:], S[:], t[:])
            nc.scalar.activation(t[:], S[:], AF.Sigmoid, scale=GELU_K)
            nc.vector.tensor_mul(S[:], S[:], t[:])

            out_v = out_d[:].rearrange("(t p) o -> t p o", p=P)
            for mt in range(c.MT):
                nc.sync.dma_start(out_v[mt], S[:, mt:mt + 1])
    return nc


FP8 = ml_dtypes.float8_e4m3fn


def prep_w_fp8(weight: np.ndarray, bias: np.ndarray, cfg: Cfg):
    """-> (wq [NQ,P,KT2,2,NQW] e4m3 of W*W_SCALE, biasr [P,N] fp32 of
    bias*W_SCALE replicated)."""
    c = cfg
    wb = (weight * W_SCALE).astype(FP8)  # [N, K]
    wq = np.ascontiguousarray(
        wb.reshape(c.NQ, c.NQW, c.KT2, 2, P).transpose(0, 4, 2, 3, 1)
    )
    biasr = np.ascontiguousarray(
        np.broadcast_to((bias * W_SCALE).astype(np.float32), (P, c.N))
    )
    return wq, biasr


def prep_x_fp8(xs: np.ndarray, cfg: Cfg) -> np.ndarray:
    """[MS, K] fp32 shard -> [MP, P, KT2, 2, XW] e4m3."""
    c = cfg
    xb = xs.astype(FP8)
    return np.ascontiguousarray(
        xb.reshape(c.MP, c.XW, c.KT2, 2, P).transpose(0, 4, 2, 3, 1)
    )


def prep_w(weight: np.ndarray, bias: np.ndarray, cfg: Cfg) -> np.ndarray:
    """[N, K] weight + [N] bias -> [NQ, P, KT, NQW] bf16 with bias row."""
    c = cfg
    kt0 = c.K // P
    wb = np.ascontiguousarray(weight.astype(BF16))  # [N, K]
    wq = np.zeros((c.NQ, P, c.KT, c.NQW), dtype=BF16)
    # wq[nq, p, k, n] = W[nq*NQW + n, k*P + p]
    wq[:, :, :kt0, :] = (
        wb.reshape(c.NQ, c.NQW, kt0, P).transpose(0, 3, 2, 1)
    )
    wq[:, 0, kt0, :] = bias.astype(BF16).reshape(c.NQ, c.NQW)
    return wq


def prep_x_shard(xs: np.ndarray, cfg: Cfg) -> np.ndarray:
    """[MS, K] fp32 shard -> [MP, P, KT, XW] bf16 with ones row."""
    c = cfg
    kt0 = c.K // P
    xb = xs.astype(BF16)  # [MS, K]
    xt = np.zeros((c.MP, P, c.KT, c.XW), dtype=BF16)
    # xt[mp, p, k, m] = x[mp*XW + m, k*P + p]
    xt[:, :, :kt0, :] = (
        xb.reshape(c.MP, c.XW, kt0, P).transpose(0, 3, 2, 1)
    )
    xt[:, 0, kt0, :] = 1.0
    return xt


_BUILT = {}


DEFAULT_DTYPE = "fp8"  # "bf16" or "fp8"


def _get_built(cfg_key=None):
    if cfg_key is None:
        cfg = Cfg(dtype=DEFAULT_DTYPE,
                  nqw=2048 if DEFAULT_DTYPE == "fp8" else 1024)
    else:
        cfg = cfg_key
    key = (cfg.M, cfg.K, cfg.N, cfg.n_cores, cfg.dtype)
    if key not in _BUILT:
        nc = bacc.Bacc("TRN2")
        if cfg.dtype == "fp8":
            build_fp8(nc, cfg)
        else:
            build(nc, cfg)
        nc.compile()
        _BUILT[key] = (nc, cfg)
    return _BUILT[key]


def _install_ntff_hook():
    """Dev-only: register the axon NTFF profile hook that the container's
    antenv stub lacks, so trace=True works. No-op if unavailable."""
    import sys
    import types
    try:
        from antenv.axon_hooks import get_axon_ntff_profile_hook  # noqa: F401
        return
    except ImportError:
        pass
    try:
        import antenv
        from trn_agent_boot.trn_boot import _ntff_profile_via_ctypes
        mod = types.ModuleType("antenv.axon_hooks")
        holder = {}
        mod.set_axon_ntff_profile_hook = lambda h: holder.__setitem__("h", h)
        mod.get_axon_ntff_profile_hook = lambda: holder.get("h")
        sys.modules["antenv.axon_hooks"] = mod
        antenv.axon_hooks = mod
        hook = _ntff_profile_via_ctypes("/opt/axon/libaxon_pjrt.so")
        if hook is not None:
            mod.set_axon_ntff_profile_hook(hook)
    except Exception as e:  # pragma: no cover - best effort
        print(f"ntff hook install failed: {e}", file=sys.stderr)


def run(x, weight, bias, trace=False):
    """Full-input entry: shard, run on 8 cores, gather. Returns
    (out [M,1] fp32, exec_time_ns or None, trace_path or None)."""
    if trace:
        _install_ntff_hook()
    nc, cfg = _get_built()
    x = np.asarray(x, dtype=np.float32)
    weight = np.asarray(weight, dtype=np.float32)
    bias = np.asarray(bias, dtype=np.float32)

    in_maps = []
    if cfg.dtype == "fp8":
        wq, biasr = prep_w_fp8(weight, bias, cfg)
        for core in range(cfg.n_cores):
            xs = x[core * cfg.MS:(core + 1) * cfg.MS]
            in_maps.append(
                {"xt": prep_x_fp8(xs, cfg), "wq": wq, "biasr": biasr})
    else:
        wq = prep_w(weight, bias, cfg)
        for core in range(cfg.n_cores):
            xs = x[core * cfg.MS:(core + 1) * cfg.MS]
            in_maps.append({"xt": prep_x_shard(xs, cfg), "wq": wq})

    res = run_bass_kernel_spmd(
        nc, in_maps, core_ids=list(range(cfg.n_cores)), trace=trace,
    )
    out = np.concatenate([r["out"] for r in res.results], axis=0)
    trace_path = None
    if res.instructions_and_trace is not None:
        trace_path = res.instructions_and_trace[1]
    return out, res.exec_time_ns, trace_path


def kernel(x, weight, bias):
    out, _, _ = run(x, weight, bias, trace=False)
    return out
